# revision 32
# baseline (speedup 1.0000x reference)
"""Trainium2 Bass kernel for LLM adapter attention (QK-RMSNorm + dual RoPE + SDPA).

Sharding: 8 cores = (batch b, head-group hg): core c -> b = c//2, heads
hg*8..hg*8+8 (hg = c%2).  Each core computes q/k/v projections for its 8
heads on its batch, attention, and a partial o_proj over its heads.  Host
sums the two partials per batch.

v2 changes vs baseline:
  - all matmul operands fp16 (fp32r QK ran at ~2 cyc/col on HW; fp16 is
    1 cyc/col and more precise than bf16)
  - softmax sum-exp moved off the PE: DVE accumulates exp tiles (fp16),
    one ones-matmul per (head, la) broadcasts the partition sum
  - exp biased by -14 so fp16 exp tiles/sums stay in range
  - RMSNorm+RoPE elementwise chain runs on fp16 SBUF copies (DVE 2x mode)
  - o_proj PSUM->SBUF copies on the scalar engine

Per-core layouts (P = 128 partitions):
  qT/kT: [128 d, head, L]  (head_dim on partitions), fp16
  v:     [128 l, l_tile, dh]  (natural), fp16
  scoresT: [128 m, 512 l] psum tiles; exp -> fp16 SBUF; softmax over
  partitions m via DVE accumulate + ones-matmul broadcast sum.
  PV: out_T[d, l] = matmul(lhsT=v, rhs=exp) accumulated over m tiles.
  o_proj: partial[l, n] = matmul(lhsT=outT, rhs=woT) accumulated over heads.
"""

import os
import sys

import numpy as np

for _p in ("/opt/trn_rl_repo", "/root/.axon_site/_ro/trn_rl_repo"):
    if _p not in sys.path and os.path.isdir(_p):
        sys.path.insert(0, _p)

def _install_ntff_hook_shim():
    """The agent image lacks ``antenv.axon_hooks``; synthesize it and wire
    the ctypes NTFF profiling hook so trace=True works under axon."""
    try:
        import antenv.axon_hooks  # noqa: F401
        return
    except ImportError:
        pass
    import types

    try:
        import antenv
    except ImportError:
        return
    mod = types.ModuleType("antenv.axon_hooks")
    mod._hook = None
    mod.set_axon_ntff_profile_hook = lambda h: setattr(mod, "_hook", h)
    mod.get_axon_ntff_profile_hook = lambda: mod._hook
    sys.modules["antenv.axon_hooks"] = mod
    antenv.axon_hooks = mod
    try:
        from trn_agent_boot.trn_boot import _ntff_profile_via_ctypes

        hook = _ntff_profile_via_ctypes("/opt/axon/libaxon_pjrt.so")
        if hook is not None:
            mod._hook = hook
    except Exception:
        pass


_install_ntff_hook_shim()

import concourse.bass as bass  # noqa: E402
import concourse.mybir as mybir  # noqa: E402
from concourse import bacc  # noqa: E402
import concourse.bacc as _bacc_mod  # noqa: E402
from concourse.bass_utils import run_bass_kernel_spmd  # noqa: E402
from concourse.hw_specs import get_activation_tables as _orig_gat  # noqa: E402
from concourse.tile import TileContext  # noqa: E402

# All scalar-engine functions this kernel uses (Exp, Ln, Copy, Square) coexist
# in the "natural_log_exp_and_others" ACT table set, but the table-load pass
# first-fits each function into the earliest set containing it, which makes
# interleaved Exp (attention) and Ln (rmsnorm) thrash ACT_TABLE_LOADs (~1.3us
# each).  Strip our functions from the earlier sets so everything first-fits
# into the one canonical set -> a single table load for the whole kernel.
_ACT_SET = "natural_log_exp_and_others"


def _patched_gat(arch):
    tabs = {k: set(v) for k, v in _orig_gat(arch).items()}
    mine = tabs[_ACT_SET]
    for name, funcs in tabs.items():
        if name == _ACT_SET:
            break
        funcs -= mine
    return tabs


_bacc_mod.get_activation_tables = _patched_gat

B, L, D = 4, 2048, 2048
NH, DH = 16, 128
EPS = 1e-6
P = 128
HPC = 8            # heads per core
NCORES = 8
KT = D // P        # 16 k-tiles for projections
LCP = 512          # l-chunk for projections
NLCP = L // LCP    # 4
LCA = 512          # l-chunk for attention
NLCA = L // LCA    # 4
MT = L // P        # 16 key tiles
LT = L // P        # 16 l-tiles
EXP_BIAS = -6.0    # keep fp16 exp tiles in normal range (scores ~ [-6, 6])
F32 = mybir.dt.float32
F16 = mybir.dt.float16

_cache = {}


def _build_program():
    nc = bacc.Bacc(
        "TRN2",
        target_bir_lowering=False,
        debug=False,
        enable_asserts=False,
        num_devices=NCORES,
    )

    xT = nc.dram_tensor("xT", [D, L], F16, kind="ExternalInput").ap()
    wqT = nc.dram_tensor("wqT", [D, HPC * DH], F16, kind="ExternalInput").ap()
    wkT = nc.dram_tensor("wkT", [D, HPC * DH], F16, kind="ExternalInput").ap()
    wvT = nc.dram_tensor("wvT", [D, HPC * DH], F16, kind="ExternalInput").ap()
    woT = nc.dram_tensor("woT", [HPC * DH, D], F16, kind="ExternalInput").ap()
    Aq = nc.dram_tensor("Aq", [DH, L], F16, kind="ExternalInput").ap()
    Bq = nc.dram_tensor("Bq", [DH, L], F16, kind="ExternalInput").ap()
    Ak = nc.dram_tensor("Ak", [DH, L], F16, kind="ExternalInput").ap()
    Bk = nc.dram_tensor("Bk", [DH, L], F16, kind="ExternalInput").ap()
    out = nc.dram_tensor("out", [L, D], F32, kind="ExternalOutput").ap()

    xTv = xT.rearrange("(ko p) l -> p ko l", p=P)        # [128, 16, 2048]
    wqv = wqT.rearrange("(ko p) d -> p ko d", p=P)
    wkv = wkT.rearrange("(ko p) d -> p ko d", p=P)
    wvv = wvT.rearrange("(ko p) d -> p ko d", p=P)
    wov = woT.rearrange("(ho p) n -> p ho n", p=P)       # [128, 8, 2048]
    outv = out.rearrange("(lt p) n -> p lt n", p=P)      # [128, 16, 2048]

    from contextlib import ExitStack
    with ExitStack() as _st:
        tc = _st.enter_context(TileContext(nc))
        constp = _st.enter_context(tc.tile_pool(name="const", bufs=1))
        xsp = _st.enter_context(tc.tile_pool(name="xs", bufs=2))
        wqkp = _st.enter_context(tc.tile_pool(name="wqk", bufs=1))
        wvp = _st.enter_context(tc.tile_pool(name="wv", bufs=1))
        qkp = _st.enter_context(tc.tile_pool(name="qkT", bufs=2))
        vsp = _st.enter_context(tc.tile_pool(name="vsb", bufs=2))
        abp = _st.enter_context(tc.tile_pool(name="ab", bufs=2))
        wkp = _st.enter_context(tc.tile_pool(name="work", bufs=2))
        expp = _st.enter_context(tc.tile_pool(name="expp", bufs=4))
        otp = _st.enter_context(tc.tile_pool(name="outT", bufs=1))
        wosp = _st.enter_context(tc.tile_pool(name="wos", bufs=2))
        psproj = _st.enter_context(tc.tile_pool(name="ps_proj", bufs=2, space="PSUM"))
        psred = _st.enter_context(tc.tile_pool(name="ps_red", bufs=1, space="PSUM"))
        pss = _st.enter_context(tc.tile_pool(name="ps_s", bufs=2, space="PSUM"))
        pspv = _st.enter_context(tc.tile_pool(name="ps_pv", bufs=1, space="PSUM"))
        if True:
            ones_h = constp.tile([P, P], F16, tag="ones_h")
            nc.vector.memset(ones_h[:], 1.0)
            eps_q = constp.tile([P, 1], F32, tag="eps_q")
            nc.vector.memset(eps_q[:], float(DH * EPS))
            eps_k = constp.tile([P, 1], F32, tag="eps_k")
            nc.vector.memset(eps_k[:], float(EPS))
            exp_b = constp.tile([P, 1], F32, tag="exp_b")
            nc.vector.memset(exp_b[:], EXP_BIAS)

            outT = otp.tile([P, HPC, L], F16, tag="outT")  # [d, head, l]

            for hp in range(HPC // 2):  # head pairs
                qT = qkp.tile([P, 2, L], F16, tag="qT")
                kT = qkp.tile([P, 2, L], F16, tag="kT")
                v_sb = vsp.tile([P, LT, 2 * DH], F16, tag="vsb")
                wvs = wvp.tile([P, KT, 2 * DH], F16, tag="wvs")
                dh0 = hp * 2 * DH
                wqs = wqkp.tile([P, KT, 2 * DH], F16, tag="wq")
                wks = wqkp.tile([P, KT, 2 * DH], F16, tag="wk")
                # split weight loads across DMA queues (startup latency);
                # q/k weights first -- they gate the very first matmul group
                nw = 4 if hp == 0 else 2
                for kh in range(nw):
                    ksl = slice(KT // nw * kh, KT // nw * (kh + 1))
                    nc.sync.dma_start(
                        wqs[:, ksl, :], wqv[:, ksl, dh0:dh0 + 2 * DH])
                    nc.sync.dma_start(
                        wks[:, ksl, :], wkv[:, ksl, dh0:dh0 + 2 * DH])
                for kh in range(2):
                    ksl = slice(8 * kh, 8 * (kh + 1))
                    nc.sync.dma_start(
                        wvs[:, ksl, :], wvv[:, ksl, dh0:dh0 + 2 * DH])

                # ---- projections + RMSNorm + RoPE ----
                for lc in range(NLCP):
                    ls = lc * LCP
                    xs = xsp.tile([P, KT, LCP], F16, tag="xs")
                    nx = 8 if (hp == 0 and lc == 0) else 4
                    for kq in range(nx):
                        ksl = slice(KT // nx * kq, KT // nx * (kq + 1))
                        nc.sync.dma_start(
                            xs[:, ksl, :], xTv[:, ksl, ls:ls + LCP])
                    aq_t = abp.tile([P, LCP], F16, tag="aq")
                    nc.sync.dma_start(aq_t[:], Aq[:, ls:ls + LCP])
                    bq_t = abp.tile([P, LCP], F16, tag="bq")
                    nc.sync.dma_start(bq_t[:], Bq[:, ls:ls + LCP])
                    ak_t = abp.tile([P, LCP], F16, tag="ak")
                    nc.sync.dma_start(ak_t[:], Ak[:, ls:ls + LCP])
                    bk_t = abp.tile([P, LCP], F16, tag="bk")
                    nc.sync.dma_start(bk_t[:], Bk[:, ls:ls + LCP])

                    for h2 in range(2):
                        for which in range(2):  # 0 = q, 1 = k
                            w_sl = wqs if which == 0 else wks
                            a_t = aq_t if which == 0 else ak_t
                            b_t = bq_t if which == 0 else bk_t
                            dst = (qT if which == 0 else kT)[:, h2, ls:ls + LCP]
                            ps = psproj.tile([P, LCP], F32, tag="proj")
                            for kt in range(KT):
                                nc.tensor.matmul(
                                    ps[:],
                                    lhsT=w_sl[:, kt, h2 * DH:(h2 + 1) * DH],
                                    rhs=xs[:, kt, :],
                                    start=(kt == 0), stop=(kt == KT - 1),
                                )
                            # fp16 copy of the projection for the DVE chain
                            pcp = wkp.tile([P, LCP], F16, tag="pcp")
                            nc.vector.tensor_copy(pcp[:], ps[:])
                            # partition-rotated copy (RoPE rotate_half) via
                            # SBUF->SBUF DMA (cross-partition moves)
                            pcr = wkp.tile([P, LCP], F16, tag="pcr")
                            nc.sync.dma_start(pcr[0:64, :], pcp[64:128, :])
                            nc.sync.dma_start(pcr[64:128, :], pcp[0:64, :])
                            sq = wkp.tile([P, LCP], F16, tag="sq")
                            nc.vector.tensor_mul(sq[:], pcp[:], pcp[:])
                            ss = psred.tile([P, LCP], F32, tag="red")
                            nc.tensor.matmul(ss[:], lhsT=ones_h[:], rhs=sq[:])
                            # rstd = exp(-0.5*ln(ss*scale + eps)); Ln+Exp stay
                            # in one ACT table set (no table thrash).
                            # q: 1/sqrt(sumsq + 128*eps) folds the 1/sqrt(DH)
                            # score scale; k: 1/sqrt(sumsq/128 + eps)
                            ln1 = wkp.tile([P, LCP], F32, tag="ln1")
                            if which == 0:
                                nc.scalar.activation(
                                    ln1[:], ss[:],
                                    mybir.ActivationFunctionType.Ln,
                                    bias=eps_q[:], scale=1.0)
                            else:
                                nc.scalar.activation(
                                    ln1[:], ss[:],
                                    mybir.ActivationFunctionType.Ln,
                                    bias=eps_k[:], scale=1.0 / DH)
                            rstd = wkp.tile([P, LCP], F16, tag="rstd")
                            nc.scalar.activation(
                                rstd[:], ln1[:],
                                mybir.ActivationFunctionType.Exp, scale=-0.5)
                            t1 = wkp.tile([P, LCP], F16, tag="t1")
                            nc.vector.tensor_mul(t1[:], pcp[:], a_t[:])
                            t2 = wkp.tile([P, LCP], F16, tag="t2")
                            nc.vector.tensor_mul(t2[:], pcr[:], b_t[:])
                            nc.vector.tensor_add(t1[:], t1[:], t2[:])
                            nc.vector.tensor_mul(dst, t1[:], rstd[:])

                    # v projection: x as stationary, natural layout
                    for sub in range(LCP // P):
                        lt = lc * (LCP // P) + sub
                        psv = psproj.tile([P, 2 * DH], F32, tag="proj")
                        for kt in range(KT):
                            nc.tensor.matmul(
                                psv[:],
                                lhsT=xs[:, kt, sub * P:(sub + 1) * P],
                                rhs=wvs[:, kt, :],
                                start=(kt == 0), stop=(kt == KT - 1),
                            )
                        nc.vector.tensor_copy(v_sb[:, lt, :], psv[:])

                # ---- attention for the two heads ----
                for h2 in range(2):
                    h = hp * 2 + h2
                    for la in range(NLCA):
                        qs = la * LCA
                        ps_pv = pspv.tile([P, LCA], F32, tag="pv")
                        acc2 = wkp.tile([P, 2, LCA], F16, tag="acc")
                        ex_prev = None
                        for mp in range(MT // 2):  # key-tile pairs
                            ps_s = pss.tile([P, 2, LCA], F32, tag="s")
                            for j in range(2):
                                mt = 2 * mp + j
                                nc.tensor.matmul(
                                    ps_s[:, j, :],
                                    lhsT=kT[:, h2, mt * P:(mt + 1) * P],
                                    rhs=qT[:, h2, qs:qs + LCA],
                                )
                            # one exp over both PSUM banks (FD=1024)
                            ex = expp.tile([P, 2, LCA], F16, tag="exp")
                            nc.scalar.activation(
                                ex[:], ps_s[:], mybir.ActivationFunctionType.Exp,
                                bias=exp_b[:])
                            for j in range(2):
                                mt = 2 * mp + j
                                nc.tensor.matmul(
                                    ps_pv[:],
                                    lhsT=v_sb[:, mt, h2 * DH:(h2 + 1) * DH],
                                    rhs=ex[:, j, :],
                                    start=(mt == 0), stop=(mt == MT - 1))
                            # DVE running sum of exp pairs (FD=1024)
                            if mp == 0:
                                ex_prev = ex
                            elif mp == 1:
                                nc.vector.tensor_add(acc2[:], ex_prev[:], ex[:])
                            else:
                                nc.vector.tensor_add(acc2[:], acc2[:], ex[:])
                        # stage PV out of PSUM immediately (frees the pv bank
                        # for the next la chunk without waiting on the recip
                        # chain)
                        pvc = wkp.tile([P, LCA], F16, tag="pvc")
                        nc.vector.tensor_copy(pvc[:], ps_pv[:])
                        # fold the pair dim, then broadcast the partition sum
                        # via ones-matmul
                        accf = wkp.tile([P, LCA], F16, tag="accf")
                        nc.vector.tensor_add(
                            accf[:], acc2[:, 0, :], acc2[:, 1, :])
                        ps_se = psred.tile([P, LCA], F32, tag="red")
                        nc.tensor.matmul(ps_se[:], lhsT=ones_h[:], rhs=accf[:])
                        rec = wkp.tile([P, LCA], F32, tag="rec")
                        nc.vector.reciprocal_approx_fast(rec[:], ps_se[:])
                        nc.vector.tensor_mul(
                            outT[:, h, qs:qs + LCA], pvc[:], rec[:])

            # ---- o_proj: partial[l, n] over this core's heads ----
            for nch in range(4):
                ns = nch * 512
                wos = wosp.tile([P, HPC, 512], F16, tag="wos")
                for hh in range(2):
                    nc.sync.dma_start(
                        wos[:, 4 * hh:4 * hh + 4, :],
                        wov[:, 4 * hh:4 * hh + 4, ns:ns + 512])
                for lt in range(LT):
                    pso = psproj.tile([P, 512], F32, tag="proj")
                    for h in range(HPC):
                        nc.tensor.matmul(
                            pso[:],
                            lhsT=outT[:, h, lt * P:(lt + 1) * P],
                            rhs=wos[:, h, :],
                            start=(h == 0), stop=(h == HPC - 1),
                        )
                    o_sb = wkp.tile([P, 512], F32, tag="osb")
                    nc.scalar.copy(o_sb[:], pso[:])
                    nc.sync.dma_start(outv[:, lt, ns:ns + 512], o_sb[:])

    nc.compile()
    return nc


def _host_prep(x, cos_q, sin_q, cos_k, sin_k, Wq, Wk, Wv, Wo, q_gamma, k_gamma):
    """Build the 8 per-core input maps (fp16, C-contiguous)."""
    f = np.float32
    f16 = np.float16
    sgn = np.concatenate([-np.ones(64, f), np.ones(64, f)])

    def ab(cos_b, sin_b, gamma):
        grot = np.concatenate([gamma[64:], gamma[:64]])
        A = np.ascontiguousarray((cos_b * gamma[None, :]).T).astype(f16)
        Bm = np.ascontiguousarray((sin_b * (sgn * grot)[None, :]).T).astype(f16)
        return A, Bm

    in_maps = []
    for c in range(NCORES):
        b, hg = divmod(c, 2)
        sl = slice(hg * HPC * DH, (hg + 1) * HPC * DH)
        A_q, B_q = ab(cos_q[b], sin_q[b], q_gamma)
        A_k, B_k = ab(cos_k[b], sin_k[b], k_gamma)
        in_maps.append({
            "xT": np.ascontiguousarray(x[b].T).astype(f16),
            "wqT": np.ascontiguousarray(Wq[sl, :].T).astype(f16),
            "wkT": np.ascontiguousarray(Wk[sl, :].T).astype(f16),
            "wvT": np.ascontiguousarray(Wv[sl, :].T).astype(f16),
            "woT": np.ascontiguousarray(Wo[:, sl].T).astype(f16),
            "Aq": A_q, "Bq": B_q, "Ak": A_k, "Bk": B_k,
        })
    return in_maps


last_results = None


def kernel(x, cos_q, sin_q, cos_k, sin_k, Wq, Wk, Wv, Wo, q_gamma, k_gamma):
    global last_results
    if "nc" not in _cache:
        _cache["nc"] = _build_program()
    nc = _cache["nc"]
    args = [np.asarray(a) for a in (x, cos_q, sin_q, cos_k, sin_k,
                                    Wq, Wk, Wv, Wo, q_gamma, k_gamma)]
    in_maps = _host_prep(*args)
    trace = bool(int(os.environ.get("BASS_KERNEL_TRACE", "0")))
    try:
        res = run_bass_kernel_spmd(
            nc, in_maps, core_ids=list(range(NCORES)), trace=trace)
    except Exception:
        if not trace:
            raise
        res = run_bass_kernel_spmd(
            nc, in_maps, core_ids=list(range(NCORES)), trace=False)
    last_results = res
    outs = [r["out"] for r in res.results]
    full = np.empty((B, L, D), np.float32)
    for b in range(B):
        full[b] = outs[2 * b] + outs[2 * b + 1]
    return full
